# revision 13
# baseline (speedup 1.0000x reference)
"""DendriticAttentionNeuron fused Bass/Tile kernel for Trainium2 (8 NeuronCores).

Strategy: data-parallel over batch (1024 rows/core, zero collectives), with
mixed-precision matmuls. Per-head scales span 16x (invtau = 1/tau, tau
log-spaced 2..32), so the output is dominated by the large-invtau heads:
heads 0-3 (o-chunks c<2) stay bf16 while heads 4-15 run in fp8-e4m3 with
perf_mode=DoubleRow (2 contraction rows per PE cell per cycle). The gate
planes are fp8 for all heads (sigmoid attenuates their error).

Layout: transposed activations yT[out_unit, batch]; per-head constants are
per-partition scalars. Quantization scales (x*16, w*2048, v*32, wout*2048)
are exact powers of two folded into the activation scale tables, so bf16 and
fp8 contributions accumulate in consistent units everywhere.

  phase Q/K: preact chains; c<2 ff/ctx bf16 (32 mm), else fp8 DoubleRow
             (16 mm); q/k stored bf16 (c<2) / e4m3 (c>=2); scores via
             0/1-mask matmuls of q*k
  phase V:   relu(32*invtau*preact), same split
  softmax:   fp32 over the 16 heads (issued after V's matmuls so the PE
             stream stays dense while ACT/DVE do the softmax)
  phase O:   out = comb.T @ (wout*2048); co<2 bf16, co>=2 fp8 DoubleRow,
             uniform 65536x PSUM scale descaled in the spike epilogue

Measured fp32-reference max-rel-err ~1.1e-2 (gate 2e-2).
"""
import numpy as np
import ml_dtypes

B, IN, H, HD = 8192, 4096, 16, 64
HID = H * HD            # 1024
N_CORES = 8
BPC = B // N_CORES      # 1024 rows per core
P = 128
NI = IN // P            # 32 i-chunks
NO = HID // P           # 8 o-chunks
NBH = 2                 # batch halves of 512 (matmul free dim)
BH = BPC // NBH         # 512
NJS = IN // 512         # 8 output j-slices
NBC = BPC // P          # 8 batch chunks of 128
NIH = NI // 2           # 16 i-chunks per half-slab
NBF_C = 2               # o-chunks kept bf16 (heads 0..3)

SX = 16.0               # x fp8 scale
SW = 2048.0             # weight fp8 scale
SXW = SX * SW           # 32768: fp8 gemm PSUM scale
SC = 32.0               # v storage scale
SO = SC * SW            # 65536: O-phase PSUM scale

TAU_MIN, TAU_MAX, TAU_SOMA = 2.0, 32.0, 2.0
V_TH = 1.0
SURROGATE_ALPHA = 4.0

W_NAMES = ["Wq_ff", "Wq_gate", "Wq_ctx", "Wk_ff", "Wk_gate", "Wk_ctx", "Wv_ff"]
BF_PLANES = [0, 2, 3, 5, 6]   # ff/ctx/v planes have a bf16 part for c < NBF_C

_CACHE = {}


def _build_nc(score_scale: float):
    import concourse.mybir as mybir
    import concourse.tile as tile
    from concourse import bacc

    bf16 = mybir.dt.bfloat16
    f8 = mybir.dt.float8e4
    f32 = mybir.dt.float32
    AF = mybir.ActivationFunctionType
    OP = mybir.AluOpType
    DR = mybir.MatmulPerfMode.DoubleRow

    nc = bacc.Bacc("TRN2", target_bir_lowering=False, debug=False,
                   num_devices=N_CORES)

    # x pre-transposed on host: xT[ci, p, b] = x[b, ci*128+p]
    xTb_d = nc.dram_tensor("xTb", [NI, P, BPC], bf16, kind="ExternalInput").ap()
    xT8_d = nc.dram_tensor("xT8", [NI, P, BPC], f8, kind="ExternalInput").ap()
    # weights pre-swizzled on host: w[j, c, p, ic, o] = W_j[ic*128+p, c*128+o]
    wcatb_d = nc.dram_tensor("wcatb", [len(BF_PLANES), NBF_C, P, NI, P], bf16,
                             kind="ExternalInput").ap()
    wcat8_d = nc.dram_tensor("wcat8", [7, NO, P, NI, P], f8,
                             kind="ExternalInput").ap()
    woutb_d = nc.dram_tensor("woutb", [NBF_C, P, IN], bf16,
                             kind="ExternalInput").ap()
    wout8_d = nc.dram_tensor("wout8", [NO - NBF_C, P, IN], f8,
                             kind="ExternalInput").ap()
    ident_d = nc.dram_tensor("ident", [P, P], f32, kind="ExternalInput").ap()
    maskS_d = nc.dram_tensor("maskS", [P, NO, H], bf16, kind="ExternalInput").ap()
    maskE_d = nc.dram_tensor("maskE", [H, NO, P], bf16, kind="ExternalInput").ap()
    tscale_d = nc.dram_tensor("tscale", [P, NO], f32, kind="ExternalInput").ap()
    vscale_d = nc.dram_tensor("vscale", [P, NO], f32, kind="ExternalInput").ap()
    bq_d = nc.dram_tensor("bq", [P, NO], f32, kind="ExternalInput").ap()
    bk_d = nc.dram_tensor("bk", [P, NO], f32, kind="ExternalInput").ap()

    spike_d = nc.dram_tensor("spike", [BPC, IN], bf16, kind="ExternalOutput").ap()
    vnew_d = nc.dram_tensor("vnew", [BPC, IN], bf16, kind="ExternalOutput").ap()

    with tile.TileContext(nc) as tc:
        with (
            tc.tile_pool(name="const", bufs=1) as cpool,
            tc.tile_pool(name="bigx", bufs=1) as bigx,
            tc.tile_pool(name="act", bufs=1) as act,
            tc.tile_pool(name="wtsb", bufs=3) as wtsb,
            tc.tile_pool(name="wts8", bufs=5) as wts8,
            tc.tile_pool(name="wopb", bufs=2) as wopb,
            tc.tile_pool(name="wop8", bufs=6) as wop8,
            tc.tile_pool(name="tmp", bufs=4) as tmp,
            tc.tile_pool(name="prodp", bufs=2) as prodp,
            tc.tile_pool(name="outp", bufs=4) as outp,
            tc.tile_pool(name="smp", bufs=4) as smp,
            tc.tile_pool(name="ps", bufs=6, space="PSUM") as ps,
            tc.tile_pool(name="pssc", bufs=1, space="PSUM") as pssc,
        ):
            # ---- constants (gpsimd queue: keep sync free for weight slabs) ----
            ident = cpool.tile([P, P], f32)
            nc.gpsimd.dma_start(out=ident[:], in_=ident_d[:])
            maskS = cpool.tile([P, NO, H], bf16)
            nc.gpsimd.dma_start(out=maskS[:], in_=maskS_d[:])
            maskE = cpool.tile([H, NO, P], bf16)
            nc.gpsimd.dma_start(out=maskE[:], in_=maskE_d[:])
            tscale = cpool.tile([P, NO], f32)
            nc.gpsimd.dma_start(out=tscale[:], in_=tscale_d[:])
            vscale = cpool.tile([P, NO], f32)
            nc.gpsimd.dma_start(out=vscale[:], in_=vscale_d[:])
            bq = cpool.tile([P, NO], f32)
            nc.gpsimd.dma_start(out=bq[:], in_=bq_d[:])
            bk = cpool.tile([P, NO], f32)
            nc.gpsimd.dma_start(out=bk[:], in_=bk_d[:])
            spike_bias = cpool.tile([P, 1], f32)
            nc.vector.memset(spike_bias[:], float(-SURROGATE_ALPHA * V_TH))

            # ---- load x: fp8 first (the early all-fp8 chunks only need it),
            # bf16 streams in behind ----
            xTb = bigx.tile([P, NI, BPC], bf16)   # 64 KiB/partition
            xT8 = bigx.tile([P, NI, BPC], f8)     # 32 KiB/partition
            for ci in range(NI):
                nc.scalar.dma_start(out=xT8[:, ci, :], in_=xT8_d[ci])
            for ci in range(NI):
                nc.scalar.dma_start(out=xTb[:, ci, :], in_=xTb_d[ci])

            # activation stores: bf16 part [P, NBF_C, BPC], fp8 part rest
            qTb = act.tile([P, NBF_C, BPC], bf16, tag="qb", bufs=2, name="qTb")
            kTb = act.tile([P, NBF_C, BPC], bf16, tag="qb", bufs=2, name="kTb")
            qT8 = act.tile([P, NO - NBF_C, BPC], f8, tag="q8", bufs=2, name="qT8")
            kT8 = act.tile([P, NO - NBF_C, BPC], f8, tag="q8", bufs=2, name="kT8")
            vTb = act.tile([P, NBF_C, BPC], bf16, tag="vb", bufs=1, name="vTb")
            vT8 = act.tile([P, NO - NBF_C, BPC], f8, tag="v8", bufs=1, name="vT8")

            scores = pssc.tile([H, BPC], f32, tag="sc")  # 2 PSUM banks

            # all-fp8 chunks first: compute starts before the bf16 x arrives
            C_ORDER = list(range(NBF_C, NO)) + list(range(NBF_C))

            def gemm_phase(planes, epi, first_phase=False):
                """planes: list of (j, jb); jb=None -> fp8 for all c."""
                np_ = len(planes)
                for ci_idx, c in enumerate(C_ORDER):
                    accs = [[ps.tile([P, BH], f32, tag="mm", bufs=6,
                                     name=f"acc{jj}{bh}") for bh in range(NBH)]
                            for jj in range(np_)]
                    modes = [(jb is not None and c < NBF_C)
                             for (j, jb) in planes]
                    for half in range(2):
                        slabs = []
                        for jj, (j, jb) in enumerate(planes):
                            if modes[jj]:
                                wt = wtsb.tile([P, NIH, P], bf16, tag="wb",
                                               bufs=3, name="wtb")
                                src = wcatb_d[jb, c, :,
                                              half * NIH:(half + 1) * NIH, :]
                            else:
                                wt = wts8.tile([P, NIH, P], f8, tag="w8",
                                               bufs=5, name="wt8")
                                src = wcat8_d[j, c, :,
                                              half * NIH:(half + 1) * NIH, :]
                            if first_phase and ci_idx == 0 and half == 0:
                                for q4 in range(4):
                                    qs = slice(q4 * (NIH // 4),
                                               (q4 + 1) * (NIH // 4))
                                    nc.sync.dma_start(out=wt[:, qs, :],
                                                      in_=src[:, qs, :])
                            else:
                                nc.sync.dma_start(out=wt[:], in_=src)
                            slabs.append(wt)
                        for t in range(NIH // 2):
                            for sub in range(2):
                                il = 2 * t + sub
                                i = half * NIH + il
                                for jj in range(np_):
                                    if modes[jj]:
                                        for bh in range(NBH):
                                            nc.tensor.matmul(
                                                accs[jj][bh][:],
                                                slabs[jj][:, il, :],
                                                xTb[:, i, bh * BH:(bh + 1) * BH],
                                                start=(i == 0),
                                                stop=(i == NI - 1))
                            i0 = half * NIH + 2 * t
                            for jj in range(np_):
                                if not modes[jj]:
                                    for bh in range(NBH):
                                        nc.tensor.matmul(
                                            accs[jj][bh][:],
                                            slabs[jj][:, 2 * t:2 * t + 2, :],
                                            xT8[:, i0:i0 + 2,
                                                bh * BH:(bh + 1) * BH],
                                            start=(i0 == 0),
                                            stop=(i0 + 2 == NI),
                                            perf_mode=DR)
                    for bh in range(NBH):
                        epi(c, bh, [accs[jj][bh] for jj in range(np_)])

            # ---- phase Q ----
            def make_qk_epi(dstb, dst8, bias, with_scores):
                def epi(c, bh, acc):
                    ff, gg, cc = acc
                    sl = slice(bh * BH, (bh + 1) * BH)
                    sig = tmp.tile([P, BH], f32, tag="t", bufs=4, name="sig")
                    nc.scalar.activation(sig[:], gg[:], AF.Sigmoid,
                                         bias=bias[:, c:c + 1],
                                         scale=float(1.0 / SXW))
                    t1 = tmp.tile([P, BH], f32, tag="t", bufs=4, name="t1")
                    nc.vector.tensor_tensor(out=t1[:], in0=sig[:], in1=cc[:],
                                            op=OP.mult)
                    t2 = tmp.tile([P, BH], f32, tag="t", bufs=4, name="t2")
                    nc.vector.tensor_tensor(out=t2[:], in0=t1[:], in1=ff[:],
                                            op=OP.add)
                    dst = dstb[:, c, sl] if c < NBF_C else dst8[:, c - NBF_C, sl]
                    nc.scalar.activation(dst, t2[:], AF.Tanh,
                                         scale=tscale[:, c:c + 1])
                    if with_scores:
                        qsrc = (qTb[:, c, sl] if c < NBF_C
                                else qT8[:, c - NBF_C, sl])
                        ksrc = (kTb[:, c, sl] if c < NBF_C
                                else kT8[:, c - NBF_C, sl])
                        prod = prodp.tile([P, BH], bf16, tag="p", bufs=2,
                                          name="prod")
                        nc.vector.tensor_tensor(out=prod[:], in0=qsrc,
                                                in1=ksrc, op=OP.mult)
                        nc.tensor.matmul(scores[:, sl], maskS[:, c, :],
                                         prod[:], start=(c == C_ORDER[0]),
                                         stop=(c == C_ORDER[-1]))
                return epi

            gemm_phase([(0, 0), (1, None), (2, 1)],
                       make_qk_epi(qTb, qT8, bq, False), first_phase=True)

            # ---- phase K (+ score accumulation) ----
            gemm_phase([(3, 2), (4, None), (5, 3)],
                       make_qk_epi(kTb, kT8, bk, True))

            # ---- softmax over heads (PE transposes run before V's matmuls,
            # the ACT/DVE chain overlaps them) ----
            scores_sb = smp.tile([H, BPC], f32, tag="ssb", bufs=1)
            nc.scalar.activation(scores_sb[:], scores[:], AF.Copy,
                                 scale=float(score_scale))
            attnT = smp.tile([H, BPC], bf16, tag="att", bufs=1)
            for bt in range(NBC):
                sl = slice(bt * P, (bt + 1) * P)
                tp = ps.tile([P, H], f32, tag="mm", bufs=6, name="tp")
                nc.tensor.transpose(tp[:], scores_sb[:, sl], ident[:H, :H])
                ex = smp.tile([P, H], f32, tag="sm", bufs=4, name="ex")
                nc.scalar.activation(ex[:], tp[:], AF.Exp)
                ssum = smp.tile([P, 1], f32, tag="sms", bufs=4, name="ssum")
                nc.vector.reduce_sum(out=ssum[:], in_=ex[:],
                                     axis=mybir.AxisListType.X)
                rec = smp.tile([P, 1], f32, tag="sms", bufs=4, name="rec")
                nc.vector.reciprocal(rec[:], ssum[:])
                at = smp.tile([P, H], f32, tag="sm", bufs=4, name="at")
                nc.vector.tensor_scalar_mul(at[:], ex[:], rec[:])
                tp2 = ps.tile([H, P], f32, tag="mm", bufs=6, name="tp2")
                nc.tensor.transpose(tp2[:], at[:], ident[:])
                nc.vector.tensor_copy(attnT[:, sl], tp2[:])

            # ---- phase V, with attn-expand + combine fused into the
            # epilogue so comb is ready the moment the last V matmul ends ----
            combb = act.tile([P, NBF_C, BPC], bf16, tag="qb", bufs=2,
                             name="combb")
            comb8 = act.tile([P, NO - NBF_C, BPC], f8, tag="q8", bufs=2,
                             name="comb8")

            def epi_v(c, bh, acc):
                sl = slice(bh * BH, (bh + 1) * BH)
                dst = vTb[:, c, sl] if c < NBF_C else vT8[:, c - NBF_C, sl]
                nc.scalar.activation(dst, acc[0][:], AF.Relu,
                                     scale=vscale[:, c:c + 1])
                exp_ps = ps.tile([P, BH], f32, tag="mm", bufs=6, name="expps")
                nc.tensor.matmul(exp_ps[:], maskE[:, c, :], attnT[:, sl],
                                 start=True, stop=True)
                cdst = (combb[:, c, sl] if c < NBF_C
                        else comb8[:, c - NBF_C, sl])
                nc.vector.tensor_tensor(out=cdst, in0=exp_ps[:], in1=dst,
                                        op=OP.mult)

            def load_wo(jp, eng):
                wob = wopb.tile([P, 2, 1024], bf16, tag="wob", bufs=2,
                                name="wob")
                for col in range(2):
                    eng.dma_start(
                        out=wob[:, col, :],
                        in_=woutb_d[col, :, jp * 1024:(jp + 1) * 1024])
                wo8s = []
                for tp8 in range(3):
                    wo8 = wop8.tile([P, 2, 1024], f8, tag="wo8", bufs=6,
                                    name="wo8")
                    for col in range(2):
                        eng.dma_start(
                            out=wo8[:, col, :],
                            in_=wout8_d[tp8 * 2 + col, :,
                                        jp * 1024:(jp + 1) * 1024])
                    wo8s.append(wo8)
                return wob, wo8s

            # jp=0 wout prefetch on the idle gpsimd queue, before phase V
            wo_tiles = load_wo(0, nc.gpsimd)

            gemm_phase([(6, 4)], epi_v)

            # ---- phase O: output projection + spike/v_new ----
            for jp in range(NJS // 2):
                wob, wo8s = wo_tiles
                if jp + 1 < NJS // 2:
                    wo_tiles = load_wo(jp + 1, nc.sync)
                for bc in range(NBC):
                    bsl = slice(bc * P, (bc + 1) * P)
                    pos = [ps.tile([P, 512], f32, tag="mm", bufs=6,
                                   name=f"po{h}") for h in range(2)]
                    for step in range(5):
                        for h in range(2):
                            hsl = slice(h * 512, (h + 1) * 512)
                            if step < NBF_C:
                                nc.tensor.matmul(
                                    pos[h][:], combb[:, step, bsl],
                                    wob[:, step, hsl],
                                    start=(step == 0), stop=False)
                            else:
                                tp8 = step - NBF_C
                                nc.tensor.matmul(
                                    pos[h][:],
                                    comb8[:, 2 * tp8:2 * tp8 + 2, bsl],
                                    wo8s[tp8][:, :, hsl],
                                    start=False, stop=(step == 4),
                                    perf_mode=DR)
                    for h in range(2):
                        js = jp * 2 + h
                        po = pos[h]
                        spk = outp.tile([P, 512], bf16, tag="o", bufs=4,
                                        name="spk")
                        nc.scalar.activation(
                            spk[:], po[:], AF.Sigmoid,
                            scale=float(SURROGATE_ALPHA / (TAU_SOMA * SO)),
                            bias=spike_bias[:])
                        vnw = outp.tile([P, 512], bf16, tag="o", bufs=4,
                                        name="vnw")
                        nc.vector.scalar_tensor_tensor(
                            out=vnw[:], in0=po[:],
                            scalar=float(1.0 / (TAU_SOMA * SO)),
                            in1=spk[:], op0=OP.mult, op1=OP.subtract)
                        nc.scalar.dma_start(
                            out=spike_d[bsl, js * 512:(js + 1) * 512],
                            in_=spk[:])
                        nc.gpsimd.dma_start(
                            out=vnew_d[bsl, js * 512:(js + 1) * 512],
                            in_=vnw[:])

    nc.finalize()
    return nc


def _host_consts():
    bf16 = ml_dtypes.bfloat16
    taus = np.logspace(np.log10(TAU_MIN), np.log10(TAU_MAX), H).astype(np.float32)
    inv_tau = 1.0 / taus                       # [H]
    pidx = np.arange(P)
    ident = np.eye(P, dtype=np.float32)
    maskS = np.zeros((P, NO, H), dtype=np.float32)
    maskE = np.zeros((H, NO, P), dtype=np.float32)
    invtau_pk = np.zeros((P, NO), dtype=np.float32)
    for c in range(NO):
        heads = 2 * c + pidx // HD             # [P] global head index
        maskS[pidx, c, heads] = 1.0
        maskE[heads, c, pidx] = 1.0
        invtau_pk[:, c] = inv_tau[heads]
    return ident, maskS.astype(bf16), maskE.astype(bf16), invtau_pk


def _pack_bias(b):  # b: [H, HD] -> [P, NO]
    out = np.zeros((P, NO), dtype=np.float32)
    pidx = np.arange(P)
    for c in range(NO):
        heads = 2 * c + pidx // HD
        out[:, c] = np.asarray(b, np.float32)[heads, pidx % HD]
    return out


def kernel(**inputs):
    from concourse.bass_utils import run_bass_kernel_spmd

    bf16 = ml_dtypes.bfloat16
    e4 = ml_dtypes.float8_e4m3
    x = np.ascontiguousarray(np.asarray(inputs["x"], dtype=np.float32))
    temperature = float(np.asarray(inputs["temperature"], dtype=np.float32))
    score_scale = 1.0 / (np.sqrt(HD) * temperature)

    key = round(score_scale, 12)
    if key not in _CACHE:
        _CACHE[key] = _build_nc(score_scale)
    nc = _CACHE[key]

    # weights: [H, IN, HD] -> swizzle to [NO, P, NI, P]:
    #   w[j, c, p, ic, o] = Wj[ic*128+p, c*128+o]
    wall = np.stack([
        np.asarray(inputs[n], np.float32).transpose(1, 0, 2).reshape(IN, HID)
          .reshape(NI, P, NO, P).transpose(2, 1, 0, 3)
        for n in W_NAMES
    ])                                          # [7, NO, P, NI, P] f32
    wcat8 = np.ascontiguousarray((wall * SW).astype(e4))
    wcatb = np.ascontiguousarray(wall[BF_PLANES][:, :NBF_C].astype(bf16))
    wout = np.asarray(inputs["Wout"], np.float32) * SW      # [HID, IN]
    woutb = np.ascontiguousarray(
        wout[:NBF_C * P].reshape(NBF_C, P, IN).astype(bf16))
    wout8 = np.ascontiguousarray(
        wout[NBF_C * P:].reshape(NO - NBF_C, P, IN).astype(e4))

    ident, maskS, maskE, invtau_pk = _host_consts()
    tscale = invtau_pk.copy()
    tscale[:, NBF_C:] /= SXW
    vscale = SC * tscale
    bq = _pack_bias(inputs["bq_gate"])
    bk = _pack_bias(inputs["bk_gate"])

    xr = x.reshape(N_CORES, BPC, IN)
    in_maps = []
    for c in range(N_CORES):
        xT = np.ascontiguousarray(xr[c].T).reshape(NI, P, BPC)
        in_maps.append({
            "xTb": xT.astype(bf16),
            "xT8": (xT * np.float32(SX)).astype(e4),
            "wcatb": wcatb, "wcat8": wcat8,
            "woutb": woutb, "wout8": wout8,
            "ident": ident, "maskS": maskS, "maskE": maskE,
            "tscale": tscale, "vscale": vscale, "bq": bq, "bk": bk,
        })

    res = run_bass_kernel_spmd(nc, in_maps, list(range(N_CORES)))
    kernel.last_results = res
    spike = np.concatenate(
        [np.asarray(res.results[c]["spike"], dtype=np.float32)
         for c in range(N_CORES)], axis=0)
    vnew = np.concatenate(
        [np.asarray(res.results[c]["vnew"], dtype=np.float32)
         for c in range(N_CORES)], axis=0)
    return (spike, vnew)


# revision 14
# speedup vs baseline: 1.1630x; 1.1630x over previous
"""DendriticAttentionNeuron fused Bass/Tile kernel for Trainium2 (8 NeuronCores).

Strategy: data-parallel over batch (1024 rows/core, zero collectives), with
mixed-precision matmuls. Per-head scales span 16x (invtau = 1/tau, tau
log-spaced 2..32), so the output is dominated by the large-invtau heads:
heads 0-3 (o-chunks c<2) stay bf16 while heads 4-15 run in fp8-e4m3 with
perf_mode=DoubleRow (2 contraction rows per PE cell per cycle). The gate
planes are fp8 for all heads (sigmoid attenuates their error).

Layout: transposed activations yT[out_unit, batch]; per-head constants are
per-partition scalars. Quantization scales (x*16, w*2048, v*32, wout*2048)
are exact powers of two folded into the activation scale tables, so bf16 and
fp8 contributions accumulate in consistent units everywhere.

  phase Q/K: preact chains; c<2 ff/ctx bf16 (32 mm), else fp8 DoubleRow
             (16 mm); q/k stored bf16 (c<2) / e4m3 (c>=2); scores via
             0/1-mask matmuls of q*k
  phase V:   relu(32*invtau*preact), same split
  softmax:   fp32 over the 16 heads (issued after V's matmuls so the PE
             stream stays dense while ACT/DVE do the softmax)
  phase O:   out = comb.T @ (wout*2048); co<2 bf16, co>=2 fp8 DoubleRow,
             uniform 65536x PSUM scale descaled in the spike epilogue

Measured fp32-reference max-rel-err ~1.1e-2 (gate 2e-2).
"""
import numpy as np
import ml_dtypes

B, IN, H, HD = 8192, 4096, 16, 64
HID = H * HD            # 1024
N_CORES = 8
BPC = B // N_CORES      # 1024 rows per core
P = 128
NI = IN // P            # 32 i-chunks
NO = HID // P           # 8 o-chunks
NBH = 2                 # batch halves of 512 (matmul free dim)
BH = BPC // NBH         # 512
NJS = IN // 512         # 8 output j-slices
NBC = BPC // P          # 8 batch chunks of 128
NIH = NI // 2           # 16 i-chunks per half-slab
NBF_C = 2               # o-chunks kept bf16 (heads 0..3)

SX = 16.0               # x fp8 scale
SW = 2048.0             # weight fp8 scale
SXW = SX * SW           # 32768: fp8 gemm PSUM scale
SC = 32.0               # v storage scale
SO = SC * SW            # 65536: O-phase PSUM scale

TAU_MIN, TAU_MAX, TAU_SOMA = 2.0, 32.0, 2.0
V_TH = 1.0
SURROGATE_ALPHA = 4.0

W_NAMES = ["Wq_ff", "Wq_gate", "Wq_ctx", "Wk_ff", "Wk_gate", "Wk_ctx", "Wv_ff"]
BF_PLANES = [0, 2, 3, 5, 6]   # ff/ctx/v planes have a bf16 part for c < NBF_C

_CACHE = {}


def _build_nc(score_scale: float):
    import concourse.mybir as mybir
    import concourse.tile as tile
    from concourse import bacc

    bf16 = mybir.dt.bfloat16
    f8 = mybir.dt.float8e4
    f32 = mybir.dt.float32
    AF = mybir.ActivationFunctionType
    OP = mybir.AluOpType
    DR = mybir.MatmulPerfMode.DoubleRow

    nc = bacc.Bacc("TRN2", target_bir_lowering=False, debug=False,
                   num_devices=N_CORES)

    # x pre-transposed on host: xT[ci, p, b] = x[b, ci*128+p]
    xTb_d = nc.dram_tensor("xTb", [NI, P, BPC], bf16, kind="ExternalInput").ap()
    xT8_d = nc.dram_tensor("xT8", [NI, P, BPC], f8, kind="ExternalInput").ap()
    # weights pre-swizzled on host: w[j, c, p, ic, o] = W_j[ic*128+p, c*128+o]
    wcatb_d = nc.dram_tensor("wcatb", [len(BF_PLANES), NBF_C, P, NI, P], bf16,
                             kind="ExternalInput").ap()
    wcat8_d = nc.dram_tensor("wcat8", [7, NO, P, NI, P], f8,
                             kind="ExternalInput").ap()
    woutb_d = nc.dram_tensor("woutb", [NBF_C, P, IN], bf16,
                             kind="ExternalInput").ap()
    wout8_d = nc.dram_tensor("wout8", [NO - NBF_C, P, IN], f8,
                             kind="ExternalInput").ap()
    ident_d = nc.dram_tensor("ident", [P, P], f32, kind="ExternalInput").ap()
    maskS_d = nc.dram_tensor("maskS", [P, NO, H], bf16, kind="ExternalInput").ap()
    maskE_d = nc.dram_tensor("maskE", [H, NO, P], bf16, kind="ExternalInput").ap()
    tscale_d = nc.dram_tensor("tscale", [P, NO], f32, kind="ExternalInput").ap()
    vscale_d = nc.dram_tensor("vscale", [P, NO], f32, kind="ExternalInput").ap()
    bq_d = nc.dram_tensor("bq", [P, NO], f32, kind="ExternalInput").ap()
    bk_d = nc.dram_tensor("bk", [P, NO], f32, kind="ExternalInput").ap()

    spike_d = nc.dram_tensor("spike", [BPC, IN], bf16, kind="ExternalOutput").ap()
    vnew_d = nc.dram_tensor("vnew", [BPC, IN], bf16, kind="ExternalOutput").ap()

    with tile.TileContext(nc) as tc:
        with (
            tc.tile_pool(name="const", bufs=1) as cpool,
            tc.tile_pool(name="bigx", bufs=1) as bigx,
            tc.tile_pool(name="act", bufs=1) as act,
            tc.tile_pool(name="wtsb", bufs=3) as wtsb,
            tc.tile_pool(name="wts8", bufs=5) as wts8,
            tc.tile_pool(name="wopb", bufs=2) as wopb,
            tc.tile_pool(name="wop8", bufs=6) as wop8,
            tc.tile_pool(name="tmp", bufs=4) as tmp,
            tc.tile_pool(name="prodp", bufs=2) as prodp,
            tc.tile_pool(name="outp", bufs=4) as outp,
            tc.tile_pool(name="smp", bufs=4) as smp,
            tc.tile_pool(name="ps", bufs=6, space="PSUM") as ps,
            tc.tile_pool(name="pssc", bufs=1, space="PSUM") as pssc,
        ):
            # ---- constants (gpsimd queue: keep sync free for weight slabs) ----
            ident = cpool.tile([P, P], f32)
            nc.gpsimd.dma_start(out=ident[:], in_=ident_d[:])
            maskS = cpool.tile([P, NO, H], bf16)
            nc.gpsimd.dma_start(out=maskS[:], in_=maskS_d[:])
            maskE = cpool.tile([H, NO, P], bf16)
            nc.gpsimd.dma_start(out=maskE[:], in_=maskE_d[:])
            tscale = cpool.tile([P, NO], f32)
            nc.gpsimd.dma_start(out=tscale[:], in_=tscale_d[:])
            vscale = cpool.tile([P, NO], f32)
            nc.gpsimd.dma_start(out=vscale[:], in_=vscale_d[:])
            bq = cpool.tile([P, NO], f32)
            nc.gpsimd.dma_start(out=bq[:], in_=bq_d[:])
            bk = cpool.tile([P, NO], f32)
            nc.gpsimd.dma_start(out=bk[:], in_=bk_d[:])
            spike_bias = cpool.tile([P, 1], f32)
            nc.vector.memset(spike_bias[:], float(-SURROGATE_ALPHA * V_TH))

            # ---- PE warmup: free matmuls on a zeroed tile while the first
            # DMAs are in flight, so HAM reaches K=8/8 before real work ----
            warm = cpool.tile([P, P], bf16)
            nc.vector.memset(warm[:], 0.0)
            for _ in range(48):
                wps = ps.tile([P, P], f32, tag="mm", bufs=6, name="warm")
                nc.tensor.matmul(wps[:], warm[:], warm[:], start=True,
                                 stop=True)

            # ---- load x. fp8 on the scalar queue (needed first, and few
            # enough pushes not to clog the ACT ring ahead of epilogues);
            # bf16 streams on gpsimd — only needed once c<NBF_C comes up ----
            xTb = bigx.tile([P, NI, BPC], bf16)   # 64 KiB/partition
            xT8 = bigx.tile([P, NI, BPC], f8)     # 32 KiB/partition
            for ci in range(NI):
                nc.scalar.dma_start(out=xT8[:, ci, :], in_=xT8_d[ci])
            for ci in range(NI):
                nc.gpsimd.dma_start(out=xTb[:, ci, :], in_=xTb_d[ci])

            # activation stores: bf16 part [P, NBF_C, BPC], fp8 part rest
            qTb = act.tile([P, NBF_C, BPC], bf16, tag="qb", bufs=2, name="qTb")
            kTb = act.tile([P, NBF_C, BPC], bf16, tag="qb", bufs=2, name="kTb")
            qT8 = act.tile([P, NO - NBF_C, BPC], f8, tag="q8", bufs=2, name="qT8")
            kT8 = act.tile([P, NO - NBF_C, BPC], f8, tag="q8", bufs=2, name="kT8")
            vTb = act.tile([P, NBF_C, BPC], bf16, tag="vb", bufs=1, name="vTb")
            vT8 = act.tile([P, NO - NBF_C, BPC], f8, tag="v8", bufs=1, name="vT8")

            scores = pssc.tile([H, BPC], f32, tag="sc")  # 2 PSUM banks

            # all-fp8 chunks first: compute starts before the bf16 x arrives
            C_ORDER = list(range(NBF_C, NO)) + list(range(NBF_C))

            def gemm_phase(planes, epi, first_phase=False):
                """planes: list of (j, jb); jb=None -> fp8 for all c."""
                np_ = len(planes)
                for ci_idx, c in enumerate(C_ORDER):
                    accs = [[ps.tile([P, BH], f32, tag="mm", bufs=6,
                                     name=f"acc{jj}{bh}") for bh in range(NBH)]
                            for jj in range(np_)]
                    modes = [(jb is not None and c < NBF_C)
                             for (j, jb) in planes]
                    for half in range(2):
                        slabs = []
                        for jj, (j, jb) in enumerate(planes):
                            if modes[jj]:
                                wt = wtsb.tile([P, NIH, P], bf16, tag="wb",
                                               bufs=3, name="wtb")
                                src = wcatb_d[jb, c, :,
                                              half * NIH:(half + 1) * NIH, :]
                            else:
                                wt = wts8.tile([P, NIH, P], f8, tag="w8",
                                               bufs=5, name="wt8")
                                src = wcat8_d[j, c, :,
                                              half * NIH:(half + 1) * NIH, :]
                            if first_phase and ci_idx == 0 and half == 0:
                                for q4 in range(4):
                                    qs = slice(q4 * (NIH // 4),
                                               (q4 + 1) * (NIH // 4))
                                    nc.sync.dma_start(out=wt[:, qs, :],
                                                      in_=src[:, qs, :])
                            else:
                                nc.sync.dma_start(out=wt[:], in_=src)
                            slabs.append(wt)
                        for t in range(NIH // 2):
                            for sub in range(2):
                                il = 2 * t + sub
                                i = half * NIH + il
                                for jj in range(np_):
                                    if modes[jj]:
                                        for bh in range(NBH):
                                            nc.tensor.matmul(
                                                accs[jj][bh][:],
                                                slabs[jj][:, il, :],
                                                xTb[:, i, bh * BH:(bh + 1) * BH],
                                                start=(i == 0),
                                                stop=(i == NI - 1))
                            i0 = half * NIH + 2 * t
                            for jj in range(np_):
                                if not modes[jj]:
                                    for bh in range(NBH):
                                        nc.tensor.matmul(
                                            accs[jj][bh][:],
                                            slabs[jj][:, 2 * t:2 * t + 2, :],
                                            xT8[:, i0:i0 + 2,
                                                bh * BH:(bh + 1) * BH],
                                            start=(i0 == 0),
                                            stop=(i0 + 2 == NI),
                                            perf_mode=DR)
                    for bh in range(NBH):
                        epi(c, bh, [accs[jj][bh] for jj in range(np_)])

            # ---- phase Q ----
            def make_qk_epi(dstb, dst8, bias, with_scores):
                def epi(c, bh, acc):
                    ff, gg, cc = acc
                    sl = slice(bh * BH, (bh + 1) * BH)
                    sig = tmp.tile([P, BH], f32, tag="t", bufs=4, name="sig")
                    nc.scalar.activation(sig[:], gg[:], AF.Sigmoid,
                                         bias=bias[:, c:c + 1],
                                         scale=float(1.0 / SXW))
                    t1 = tmp.tile([P, BH], f32, tag="t", bufs=4, name="t1")
                    nc.vector.tensor_tensor(out=t1[:], in0=sig[:], in1=cc[:],
                                            op=OP.mult)
                    t2 = tmp.tile([P, BH], f32, tag="t", bufs=4, name="t2")
                    nc.vector.tensor_tensor(out=t2[:], in0=t1[:], in1=ff[:],
                                            op=OP.add)
                    dst = dstb[:, c, sl] if c < NBF_C else dst8[:, c - NBF_C, sl]
                    nc.scalar.activation(dst, t2[:], AF.Tanh,
                                         scale=tscale[:, c:c + 1])
                    if with_scores:
                        qsrc = (qTb[:, c, sl] if c < NBF_C
                                else qT8[:, c - NBF_C, sl])
                        ksrc = (kTb[:, c, sl] if c < NBF_C
                                else kT8[:, c - NBF_C, sl])
                        prod = prodp.tile([P, BH], bf16, tag="p", bufs=2,
                                          name="prod")
                        nc.vector.tensor_tensor(out=prod[:], in0=qsrc,
                                                in1=ksrc, op=OP.mult)
                        nc.tensor.matmul(scores[:, sl], maskS[:, c, :],
                                         prod[:], start=(c == C_ORDER[0]),
                                         stop=(c == C_ORDER[-1]))
                return epi

            gemm_phase([(0, 0), (1, None), (2, 1)],
                       make_qk_epi(qTb, qT8, bq, False), first_phase=True)

            # ---- phase K (+ score accumulation) ----
            gemm_phase([(3, 2), (4, None), (5, 3)],
                       make_qk_epi(kTb, kT8, bk, True))

            # ---- softmax over heads (PE transposes run before V's matmuls,
            # the ACT/DVE chain overlaps them) ----
            scores_sb = smp.tile([H, BPC], f32, tag="ssb", bufs=1)
            nc.scalar.activation(scores_sb[:], scores[:], AF.Copy,
                                 scale=float(score_scale))
            attnT = smp.tile([H, BPC], bf16, tag="att", bufs=1)
            for bt in range(NBC):
                sl = slice(bt * P, (bt + 1) * P)
                tp = ps.tile([P, H], f32, tag="mm", bufs=6, name="tp")
                nc.tensor.transpose(tp[:], scores_sb[:, sl], ident[:H, :H])
                ex = smp.tile([P, H], f32, tag="sm", bufs=4, name="ex")
                nc.scalar.activation(ex[:], tp[:], AF.Exp)
                ssum = smp.tile([P, 1], f32, tag="sms", bufs=4, name="ssum")
                nc.vector.reduce_sum(out=ssum[:], in_=ex[:],
                                     axis=mybir.AxisListType.X)
                rec = smp.tile([P, 1], f32, tag="sms", bufs=4, name="rec")
                nc.vector.reciprocal(rec[:], ssum[:])
                at = smp.tile([P, H], f32, tag="sm", bufs=4, name="at")
                nc.vector.tensor_scalar_mul(at[:], ex[:], rec[:])
                tp2 = ps.tile([H, P], f32, tag="mm", bufs=6, name="tp2")
                nc.tensor.transpose(tp2[:], at[:], ident[:])
                nc.vector.tensor_copy(attnT[:, sl], tp2[:])

            # ---- phase V, with attn-expand + combine fused into the
            # epilogue so comb is ready the moment the last V matmul ends ----
            combb = act.tile([P, NBF_C, BPC], bf16, tag="qb", bufs=2,
                             name="combb")
            comb8 = act.tile([P, NO - NBF_C, BPC], f8, tag="q8", bufs=2,
                             name="comb8")

            def epi_v(c, bh, acc):
                sl = slice(bh * BH, (bh + 1) * BH)
                dst = vTb[:, c, sl] if c < NBF_C else vT8[:, c - NBF_C, sl]
                nc.scalar.activation(dst, acc[0][:], AF.Relu,
                                     scale=vscale[:, c:c + 1])
                exp_ps = ps.tile([P, BH], f32, tag="mm", bufs=6, name="expps")
                nc.tensor.matmul(exp_ps[:], maskE[:, c, :], attnT[:, sl],
                                 start=True, stop=True)
                cdst = (combb[:, c, sl] if c < NBF_C
                        else comb8[:, c - NBF_C, sl])
                nc.vector.tensor_tensor(out=cdst, in0=exp_ps[:], in1=dst,
                                        op=OP.mult)

            def load_wo(jp, eng):
                wob = wopb.tile([P, 2, 1024], bf16, tag="wob", bufs=2,
                                name="wob")
                for col in range(2):
                    eng.dma_start(
                        out=wob[:, col, :],
                        in_=woutb_d[col, :, jp * 1024:(jp + 1) * 1024])
                wo8s = []
                for tp8 in range(3):
                    wo8 = wop8.tile([P, 2, 1024], f8, tag="wo8", bufs=6,
                                    name="wo8")
                    for col in range(2):
                        eng.dma_start(
                            out=wo8[:, col, :],
                            in_=wout8_d[tp8 * 2 + col, :,
                                        jp * 1024:(jp + 1) * 1024])
                    wo8s.append(wo8)
                return wob, wo8s

            # jp=0 wout prefetch on the idle gpsimd queue, before phase V
            wo_tiles = load_wo(0, nc.gpsimd)

            gemm_phase([(6, 4)], epi_v)

            # ---- phase O: output projection + spike/v_new ----
            for jp in range(NJS // 2):
                wob, wo8s = wo_tiles
                if jp + 1 < NJS // 2:
                    wo_tiles = load_wo(jp + 1, nc.sync)
                for bc in range(NBC):
                    bsl = slice(bc * P, (bc + 1) * P)
                    pos = [ps.tile([P, 512], f32, tag="mm", bufs=6,
                                   name=f"po{h}") for h in range(2)]
                    for step in range(5):
                        for h in range(2):
                            hsl = slice(h * 512, (h + 1) * 512)
                            if step < NBF_C:
                                nc.tensor.matmul(
                                    pos[h][:], combb[:, step, bsl],
                                    wob[:, step, hsl],
                                    start=(step == 0), stop=False)
                            else:
                                tp8 = step - NBF_C
                                nc.tensor.matmul(
                                    pos[h][:],
                                    comb8[:, 2 * tp8:2 * tp8 + 2, bsl],
                                    wo8s[tp8][:, :, hsl],
                                    start=False, stop=(step == 4),
                                    perf_mode=DR)
                    for h in range(2):
                        js = jp * 2 + h
                        po = pos[h]
                        spk = outp.tile([P, 512], bf16, tag="o", bufs=4,
                                        name="spk")
                        nc.scalar.activation(
                            spk[:], po[:], AF.Sigmoid,
                            scale=float(SURROGATE_ALPHA / (TAU_SOMA * SO)),
                            bias=spike_bias[:])
                        vnw = outp.tile([P, 512], bf16, tag="o", bufs=4,
                                        name="vnw")
                        nc.vector.scalar_tensor_tensor(
                            out=vnw[:], in0=po[:],
                            scalar=float(1.0 / (TAU_SOMA * SO)),
                            in1=spk[:], op0=OP.mult, op1=OP.subtract)
                        nc.scalar.dma_start(
                            out=spike_d[bsl, js * 512:(js + 1) * 512],
                            in_=spk[:])
                        nc.gpsimd.dma_start(
                            out=vnew_d[bsl, js * 512:(js + 1) * 512],
                            in_=vnw[:])

    nc.finalize()
    return nc


def _host_consts():
    bf16 = ml_dtypes.bfloat16
    taus = np.logspace(np.log10(TAU_MIN), np.log10(TAU_MAX), H).astype(np.float32)
    inv_tau = 1.0 / taus                       # [H]
    pidx = np.arange(P)
    ident = np.eye(P, dtype=np.float32)
    maskS = np.zeros((P, NO, H), dtype=np.float32)
    maskE = np.zeros((H, NO, P), dtype=np.float32)
    invtau_pk = np.zeros((P, NO), dtype=np.float32)
    for c in range(NO):
        heads = 2 * c + pidx // HD             # [P] global head index
        maskS[pidx, c, heads] = 1.0
        maskE[heads, c, pidx] = 1.0
        invtau_pk[:, c] = inv_tau[heads]
    return ident, maskS.astype(bf16), maskE.astype(bf16), invtau_pk


def _pack_bias(b):  # b: [H, HD] -> [P, NO]
    out = np.zeros((P, NO), dtype=np.float32)
    pidx = np.arange(P)
    for c in range(NO):
        heads = 2 * c + pidx // HD
        out[:, c] = np.asarray(b, np.float32)[heads, pidx % HD]
    return out


def kernel(**inputs):
    from concourse.bass_utils import run_bass_kernel_spmd

    bf16 = ml_dtypes.bfloat16
    e4 = ml_dtypes.float8_e4m3
    x = np.ascontiguousarray(np.asarray(inputs["x"], dtype=np.float32))
    temperature = float(np.asarray(inputs["temperature"], dtype=np.float32))
    score_scale = 1.0 / (np.sqrt(HD) * temperature)

    key = round(score_scale, 12)
    if key not in _CACHE:
        _CACHE[key] = _build_nc(score_scale)
    nc = _CACHE[key]

    # weights: [H, IN, HD] -> swizzle to [NO, P, NI, P]:
    #   w[j, c, p, ic, o] = Wj[ic*128+p, c*128+o]
    wall = np.stack([
        np.asarray(inputs[n], np.float32).transpose(1, 0, 2).reshape(IN, HID)
          .reshape(NI, P, NO, P).transpose(2, 1, 0, 3)
        for n in W_NAMES
    ])                                          # [7, NO, P, NI, P] f32
    wcat8 = np.ascontiguousarray((wall * SW).astype(e4))
    wcatb = np.ascontiguousarray(wall[BF_PLANES][:, :NBF_C].astype(bf16))
    wout = np.asarray(inputs["Wout"], np.float32) * SW      # [HID, IN]
    woutb = np.ascontiguousarray(
        wout[:NBF_C * P].reshape(NBF_C, P, IN).astype(bf16))
    wout8 = np.ascontiguousarray(
        wout[NBF_C * P:].reshape(NO - NBF_C, P, IN).astype(e4))

    ident, maskS, maskE, invtau_pk = _host_consts()
    tscale = invtau_pk.copy()
    tscale[:, NBF_C:] /= SXW
    vscale = SC * tscale
    bq = _pack_bias(inputs["bq_gate"])
    bk = _pack_bias(inputs["bk_gate"])

    xr = x.reshape(N_CORES, BPC, IN)
    in_maps = []
    for c in range(N_CORES):
        xT = np.ascontiguousarray(xr[c].T).reshape(NI, P, BPC)
        in_maps.append({
            "xTb": xT.astype(bf16),
            "xT8": (xT * np.float32(SX)).astype(e4),
            "wcatb": wcatb, "wcat8": wcat8,
            "woutb": woutb, "wout8": wout8,
            "ident": ident, "maskS": maskS, "maskE": maskE,
            "tscale": tscale, "vscale": vscale, "bq": bq, "bk": bk,
        })

    res = run_bass_kernel_spmd(nc, in_maps, list(range(N_CORES)))
    kernel.last_results = res
    spike = np.concatenate(
        [np.asarray(res.results[c]["spike"], dtype=np.float32)
         for c in range(N_CORES)], axis=0)
    vnew = np.concatenate(
        [np.asarray(res.results[c]["vnew"], dtype=np.float32)
         for c in range(N_CORES)], axis=0)
    return (spike, vnew)


# revision 16
# speedup vs baseline: 1.1702x; 1.0062x over previous
"""DendriticAttentionNeuron fused Bass/Tile kernel for Trainium2 (8 NeuronCores).

Strategy: data-parallel over batch (1024 rows/core, zero collectives), with
mixed-precision matmuls. Per-head scales span 16x (invtau = 1/tau, tau
log-spaced 2..32), so the output is dominated by the large-invtau heads:
heads 0-3 (o-chunks c<2) stay bf16 while heads 4-15 run in fp8-e4m3 with
perf_mode=DoubleRow (2 contraction rows per PE cell per cycle). The gate
planes are fp8 for all heads (sigmoid attenuates their error).

Layout: transposed activations yT[out_unit, batch]; per-head constants are
per-partition scalars. Quantization scales (x*16, w*2048, v*32, wout*2048)
are exact powers of two folded into the activation scale tables, so bf16 and
fp8 contributions accumulate in consistent units everywhere.

  phase Q/K: preact chains; c<2 ff/ctx bf16 (32 mm), else fp8 DoubleRow
             (16 mm); q/k stored bf16 (c<2) / e4m3 (c>=2); scores via
             0/1-mask matmuls of q*k
  phase V:   relu(32*invtau*preact), same split
  softmax:   fp32 over the 16 heads (issued after V's matmuls so the PE
             stream stays dense while ACT/DVE do the softmax)
  phase O:   out = comb.T @ (wout*2048); co<2 bf16, co>=2 fp8 DoubleRow,
             uniform 65536x PSUM scale descaled in the spike epilogue

Measured fp32-reference max-rel-err ~1.1e-2 (gate 2e-2).
"""
import numpy as np
import ml_dtypes

B, IN, H, HD = 8192, 4096, 16, 64
HID = H * HD            # 1024
N_CORES = 8
BPC = B // N_CORES      # 1024 rows per core
P = 128
NI = IN // P            # 32 i-chunks
NO = HID // P           # 8 o-chunks
NBH = 2                 # batch halves of 512 (matmul free dim)
BH = BPC // NBH         # 512
NJS = IN // 512         # 8 output j-slices
NBC = BPC // P          # 8 batch chunks of 128
NIH = NI // 2           # 16 i-chunks per half-slab
NBF_C = 2               # o-chunks kept bf16 (heads 0..3)

SX = 16.0               # x fp8 scale
SW = 2048.0             # weight fp8 scale
SXW = SX * SW           # 32768: fp8 gemm PSUM scale
SC = 32.0               # v storage scale
SO = SC * SW            # 65536: O-phase PSUM scale

TAU_MIN, TAU_MAX, TAU_SOMA = 2.0, 32.0, 2.0
V_TH = 1.0
SURROGATE_ALPHA = 4.0

W_NAMES = ["Wq_ff", "Wq_gate", "Wq_ctx", "Wk_ff", "Wk_gate", "Wk_ctx", "Wv_ff"]
BF_PLANES = [0, 2, 3, 5, 6]   # ff/ctx/v planes have a bf16 part for c < NBF_C

_CACHE = {}


def _build_nc(score_scale: float):
    import concourse.mybir as mybir
    import concourse.tile as tile
    from concourse import bacc

    bf16 = mybir.dt.bfloat16
    f8 = mybir.dt.float8e4
    f32 = mybir.dt.float32
    AF = mybir.ActivationFunctionType
    OP = mybir.AluOpType
    DR = mybir.MatmulPerfMode.DoubleRow

    nc = bacc.Bacc("TRN2", target_bir_lowering=False, debug=False,
                   num_devices=N_CORES)

    # x pre-transposed on host: xT[ci, p, b] = x[b, ci*128+p]
    xTb_d = nc.dram_tensor("xTb", [NI, P, BPC], bf16, kind="ExternalInput").ap()
    xT8_d = nc.dram_tensor("xT8", [NI, P, BPC], f8, kind="ExternalInput").ap()
    # weights pre-swizzled on host: w[j, c, p, ic, o] = W_j[ic*128+p, c*128+o]
    wcatb_d = nc.dram_tensor("wcatb", [len(BF_PLANES), NBF_C, P, NI, P], bf16,
                             kind="ExternalInput").ap()
    wcat8_d = nc.dram_tensor("wcat8", [7, NO, P, NI, P], f8,
                             kind="ExternalInput").ap()
    woutb_d = nc.dram_tensor("woutb", [NBF_C, P, IN], bf16,
                             kind="ExternalInput").ap()
    wout8_d = nc.dram_tensor("wout8", [NO - NBF_C, P, IN], f8,
                             kind="ExternalInput").ap()
    ident_d = nc.dram_tensor("ident", [P, P], f32, kind="ExternalInput").ap()
    maskS_d = nc.dram_tensor("maskS", [P, NO, H], bf16, kind="ExternalInput").ap()
    maskE_d = nc.dram_tensor("maskE", [H, NO, P], bf16, kind="ExternalInput").ap()
    tscale_d = nc.dram_tensor("tscale", [P, NO], f32, kind="ExternalInput").ap()
    vscale_d = nc.dram_tensor("vscale", [P, NO], f32, kind="ExternalInput").ap()
    bq_d = nc.dram_tensor("bq", [P, NO], f32, kind="ExternalInput").ap()
    bk_d = nc.dram_tensor("bk", [P, NO], f32, kind="ExternalInput").ap()

    spike_d = nc.dram_tensor("spike", [BPC, IN], bf16, kind="ExternalOutput").ap()
    vnew_d = nc.dram_tensor("vnew", [BPC, IN], bf16, kind="ExternalOutput").ap()

    with tile.TileContext(nc) as tc:
        with (
            tc.tile_pool(name="const", bufs=1) as cpool,
            tc.tile_pool(name="bigx", bufs=1) as bigx,
            tc.tile_pool(name="act", bufs=1) as act,
            tc.tile_pool(name="wtsb", bufs=3) as wtsb,
            tc.tile_pool(name="wts8", bufs=5) as wts8,
            tc.tile_pool(name="wopb", bufs=2) as wopb,
            tc.tile_pool(name="wop8", bufs=6) as wop8,
            tc.tile_pool(name="tmp", bufs=4) as tmp,
            tc.tile_pool(name="prodp", bufs=2) as prodp,
            tc.tile_pool(name="outp", bufs=4) as outp,
            tc.tile_pool(name="smp", bufs=4) as smp,
            tc.tile_pool(name="ps", bufs=6, space="PSUM") as ps,
            tc.tile_pool(name="pssc", bufs=1, space="PSUM") as pssc,
        ):
            # ---- constants (gpsimd queue: keep sync free for weight slabs) ----
            ident = cpool.tile([P, P], f32)
            nc.gpsimd.dma_start(out=ident[:], in_=ident_d[:])
            maskS = cpool.tile([P, NO, H], bf16)
            nc.gpsimd.dma_start(out=maskS[:], in_=maskS_d[:])
            maskE = cpool.tile([H, NO, P], bf16)
            nc.gpsimd.dma_start(out=maskE[:], in_=maskE_d[:])
            tscale = cpool.tile([P, NO], f32)
            nc.gpsimd.dma_start(out=tscale[:], in_=tscale_d[:])
            vscale = cpool.tile([P, NO], f32)
            nc.gpsimd.dma_start(out=vscale[:], in_=vscale_d[:])
            bq = cpool.tile([P, NO], f32)
            nc.gpsimd.dma_start(out=bq[:], in_=bq_d[:])
            bk = cpool.tile([P, NO], f32)
            nc.gpsimd.dma_start(out=bk[:], in_=bk_d[:])
            spike_bias = cpool.tile([P, 1], f32)
            nc.vector.memset(spike_bias[:], float(-SURROGATE_ALPHA * V_TH))

            # ---- PE warmup: free matmuls on a zeroed tile while the first
            # DMAs are in flight, so HAM reaches K=8/8 before real work ----
            warm = cpool.tile([P, P], bf16)
            nc.vector.memset(warm[:], 0.0)
            for _ in range(48):
                wps = ps.tile([P, P], f32, tag="mm", bufs=6, name="warm")
                nc.tensor.matmul(wps[:], warm[:], warm[:], start=True,
                                 stop=True)

            # ---- load x. fp8 on the scalar queue (needed first, and few
            # enough pushes not to clog the ACT ring ahead of epilogues);
            # bf16 streams on gpsimd — only needed once c<NBF_C comes up ----
            xTb = bigx.tile([P, NI, BPC], bf16)   # 64 KiB/partition
            xT8 = bigx.tile([P, NI, BPC], f8)     # 32 KiB/partition
            for ci in range(NI):
                nc.scalar.dma_start(out=xT8[:, ci, :], in_=xT8_d[ci])
            for ci in range(NI):
                nc.gpsimd.dma_start(out=xTb[:, ci, :], in_=xTb_d[ci])

            # activation stores: bf16 part [P, NBF_C, BPC], fp8 part rest
            qTb = act.tile([P, NBF_C, BPC], bf16, tag="qb", bufs=2, name="qTb")
            kTb = act.tile([P, NBF_C, BPC], bf16, tag="qb", bufs=2, name="kTb")
            qT8 = act.tile([P, NO - NBF_C, BPC], f8, tag="q8", bufs=2, name="qT8")
            kT8 = act.tile([P, NO - NBF_C, BPC], f8, tag="q8", bufs=2, name="kT8")
            vTb = act.tile([P, NBF_C, BPC], bf16, tag="vb", bufs=1, name="vTb")
            vT8 = act.tile([P, NO - NBF_C, BPC], f8, tag="v8", bufs=1, name="vT8")

            scores = pssc.tile([H, BPC], f32, tag="sc")  # 2 PSUM banks

            # phase Q runs all-fp8 chunks first: compute starts before the
            # bf16 x stream (on the slower gpsimd queue) has arrived
            Q_ORDER = list(range(NBF_C, NO)) + list(range(NBF_C))
            NAT_ORDER = list(range(NO))

            def gemm_phase(planes, epi, c_order, first_phase=False):
                """planes: list of (j, jb); jb=None -> fp8 for all c."""
                np_ = len(planes)
                for ci_idx, c in enumerate(c_order):
                    accs = [[ps.tile([P, BH], f32, tag="mm", bufs=6,
                                     name=f"acc{jj}{bh}") for bh in range(NBH)]
                            for jj in range(np_)]
                    modes = [(jb is not None and c < NBF_C)
                             for (j, jb) in planes]
                    for half in range(2):
                        slabs = []
                        for jj, (j, jb) in enumerate(planes):
                            if modes[jj]:
                                wt = wtsb.tile([P, NIH, P], bf16, tag="wb",
                                               bufs=3, name="wtb")
                                src = wcatb_d[jb, c, :,
                                              half * NIH:(half + 1) * NIH, :]
                            else:
                                wt = wts8.tile([P, NIH, P], f8, tag="w8",
                                               bufs=5, name="wt8")
                                src = wcat8_d[j, c, :,
                                              half * NIH:(half + 1) * NIH, :]
                            if first_phase and ci_idx == 0 and half == 0:
                                for q4 in range(4):
                                    qs = slice(q4 * (NIH // 4),
                                               (q4 + 1) * (NIH // 4))
                                    nc.sync.dma_start(out=wt[:, qs, :],
                                                      in_=src[:, qs, :])
                            else:
                                nc.sync.dma_start(out=wt[:], in_=src)
                            slabs.append(wt)
                        for t in range(NIH // 2):
                            for sub in range(2):
                                il = 2 * t + sub
                                i = half * NIH + il
                                for jj in range(np_):
                                    if modes[jj]:
                                        for bh in range(NBH):
                                            nc.tensor.matmul(
                                                accs[jj][bh][:],
                                                slabs[jj][:, il, :],
                                                xTb[:, i, bh * BH:(bh + 1) * BH],
                                                start=(i == 0),
                                                stop=(i == NI - 1))
                            i0 = half * NIH + 2 * t
                            for jj in range(np_):
                                if not modes[jj]:
                                    for bh in range(NBH):
                                        nc.tensor.matmul(
                                            accs[jj][bh][:],
                                            slabs[jj][:, 2 * t:2 * t + 2, :],
                                            xT8[:, i0:i0 + 2,
                                                bh * BH:(bh + 1) * BH],
                                            start=(i0 == 0),
                                            stop=(i0 + 2 == NI),
                                            perf_mode=DR)
                    for bh in range(NBH):
                        epi(c, bh, [accs[jj][bh] for jj in range(np_)])

            # ---- phase Q ----
            def make_qk_epi(dstb, dst8, bias, with_scores):
                def epi(c, bh, acc):
                    ff, gg, cc = acc
                    sl = slice(bh * BH, (bh + 1) * BH)
                    sig = tmp.tile([P, BH], f32, tag="t", bufs=4, name="sig")
                    nc.scalar.activation(sig[:], gg[:], AF.Sigmoid,
                                         bias=bias[:, c:c + 1],
                                         scale=float(1.0 / SXW))
                    t1 = tmp.tile([P, BH], f32, tag="t", bufs=4, name="t1")
                    nc.vector.tensor_tensor(out=t1[:], in0=sig[:], in1=cc[:],
                                            op=OP.mult)
                    t2 = tmp.tile([P, BH], f32, tag="t", bufs=4, name="t2")
                    nc.vector.tensor_tensor(out=t2[:], in0=t1[:], in1=ff[:],
                                            op=OP.add)
                    dst = dstb[:, c, sl] if c < NBF_C else dst8[:, c - NBF_C, sl]
                    nc.scalar.activation(dst, t2[:], AF.Tanh,
                                         scale=tscale[:, c:c + 1])
                    if with_scores:
                        qsrc = (qTb[:, c, sl] if c < NBF_C
                                else qT8[:, c - NBF_C, sl])
                        ksrc = (kTb[:, c, sl] if c < NBF_C
                                else kT8[:, c - NBF_C, sl])
                        prod = prodp.tile([P, BH], bf16, tag="p", bufs=2,
                                          name="prod")
                        nc.vector.tensor_tensor(out=prod[:], in0=qsrc,
                                                in1=ksrc, op=OP.mult)
                        nc.tensor.matmul(scores[:, sl], maskS[:, c, :],
                                         prod[:], start=(c == 0),
                                         stop=(c == NO - 1))
                return epi

            gemm_phase([(0, 0), (1, None), (2, 1)],
                       make_qk_epi(qTb, qT8, bq, False), Q_ORDER,
                       first_phase=True)

            # ---- phase K (+ score accumulation) ----
            gemm_phase([(3, 2), (4, None), (5, 3)],
                       make_qk_epi(kTb, kT8, bk, True), NAT_ORDER)

            # ---- softmax over heads (PE transposes run before V's matmuls,
            # the ACT/DVE chain overlaps them) ----
            scores_sb = smp.tile([H, BPC], f32, tag="ssb", bufs=1)
            nc.scalar.activation(scores_sb[:], scores[:], AF.Copy,
                                 scale=float(score_scale))
            attnT = smp.tile([H, BPC], bf16, tag="att", bufs=1)
            for bt in range(NBC):
                sl = slice(bt * P, (bt + 1) * P)
                tp = ps.tile([P, H], f32, tag="mm", bufs=6, name="tp")
                nc.tensor.transpose(tp[:], scores_sb[:, sl], ident[:H, :H])
                ex = smp.tile([P, H], f32, tag="sm", bufs=4, name="ex")
                nc.scalar.activation(ex[:], tp[:], AF.Exp)
                ssum = smp.tile([P, 1], f32, tag="sms", bufs=4, name="ssum")
                nc.vector.reduce_sum(out=ssum[:], in_=ex[:],
                                     axis=mybir.AxisListType.X)
                rec = smp.tile([P, 1], f32, tag="sms", bufs=4, name="rec")
                nc.vector.reciprocal(rec[:], ssum[:])
                at = smp.tile([P, H], f32, tag="sm", bufs=4, name="at")
                nc.vector.tensor_scalar_mul(at[:], ex[:], rec[:])
                tp2 = ps.tile([H, P], f32, tag="mm", bufs=6, name="tp2")
                nc.tensor.transpose(tp2[:], at[:], ident[:])
                nc.vector.tensor_copy(attnT[:, sl], tp2[:])

            # ---- phase V, with attn-expand + combine fused into the
            # epilogue so comb is ready the moment the last V matmul ends ----
            combb = act.tile([P, NBF_C, BPC], bf16, tag="qb", bufs=2,
                             name="combb")
            comb8 = act.tile([P, NO - NBF_C, BPC], f8, tag="q8", bufs=2,
                             name="comb8")

            def epi_v(c, bh, acc):
                sl = slice(bh * BH, (bh + 1) * BH)
                dst = vTb[:, c, sl] if c < NBF_C else vT8[:, c - NBF_C, sl]
                nc.scalar.activation(dst, acc[0][:], AF.Relu,
                                     scale=vscale[:, c:c + 1])
                exp_ps = ps.tile([P, BH], f32, tag="mm", bufs=6, name="expps")
                nc.tensor.matmul(exp_ps[:], maskE[:, c, :], attnT[:, sl],
                                 start=True, stop=True)
                cdst = (combb[:, c, sl] if c < NBF_C
                        else comb8[:, c - NBF_C, sl])
                nc.vector.tensor_tensor(out=cdst, in0=exp_ps[:], in1=dst,
                                        op=OP.mult)

            def load_wo(jp, eng):
                wob = wopb.tile([P, 2, 1024], bf16, tag="wob", bufs=2,
                                name="wob")
                for col in range(2):
                    eng.dma_start(
                        out=wob[:, col, :],
                        in_=woutb_d[col, :, jp * 1024:(jp + 1) * 1024])
                wo8s = []
                for tp8 in range(3):
                    wo8 = wop8.tile([P, 2, 1024], f8, tag="wo8", bufs=6,
                                    name="wo8")
                    for col in range(2):
                        eng.dma_start(
                            out=wo8[:, col, :],
                            in_=wout8_d[tp8 * 2 + col, :,
                                        jp * 1024:(jp + 1) * 1024])
                    wo8s.append(wo8)
                return wob, wo8s

            # jp=0 wout prefetch on the idle gpsimd queue, before phase V
            wo_tiles = load_wo(0, nc.gpsimd)

            gemm_phase([(6, 4)], epi_v, NAT_ORDER)

            # ---- phase O: output projection + spike/v_new ----
            for jp in range(NJS // 2):
                wob, wo8s = wo_tiles
                if jp + 1 < NJS // 2:
                    wo_tiles = load_wo(jp + 1, nc.sync)
                for bc in range(NBC):
                    bsl = slice(bc * P, (bc + 1) * P)
                    pos = [ps.tile([P, 512], f32, tag="mm", bufs=6,
                                   name=f"po{h}") for h in range(2)]
                    for step in range(5):
                        for h in range(2):
                            hsl = slice(h * 512, (h + 1) * 512)
                            if step < NBF_C:
                                nc.tensor.matmul(
                                    pos[h][:], combb[:, step, bsl],
                                    wob[:, step, hsl],
                                    start=(step == 0), stop=False)
                            else:
                                tp8 = step - NBF_C
                                nc.tensor.matmul(
                                    pos[h][:],
                                    comb8[:, 2 * tp8:2 * tp8 + 2, bsl],
                                    wo8s[tp8][:, :, hsl],
                                    start=False, stop=(step == 4),
                                    perf_mode=DR)
                    for h in range(2):
                        js = jp * 2 + h
                        po = pos[h]
                        spk = outp.tile([P, 512], bf16, tag="o", bufs=4,
                                        name="spk")
                        nc.scalar.activation(
                            spk[:], po[:], AF.Sigmoid,
                            scale=float(SURROGATE_ALPHA / (TAU_SOMA * SO)),
                            bias=spike_bias[:])
                        vnw = outp.tile([P, 512], bf16, tag="o", bufs=4,
                                        name="vnw")
                        nc.vector.scalar_tensor_tensor(
                            out=vnw[:], in0=po[:],
                            scalar=float(1.0 / (TAU_SOMA * SO)),
                            in1=spk[:], op0=OP.mult, op1=OP.subtract)
                        nc.scalar.dma_start(
                            out=spike_d[bsl, js * 512:(js + 1) * 512],
                            in_=spk[:])
                        nc.sync.dma_start(
                            out=vnew_d[bsl, js * 512:(js + 1) * 512],
                            in_=vnw[:])

    nc.finalize()
    return nc


def _host_consts():
    bf16 = ml_dtypes.bfloat16
    taus = np.logspace(np.log10(TAU_MIN), np.log10(TAU_MAX), H).astype(np.float32)
    inv_tau = 1.0 / taus                       # [H]
    pidx = np.arange(P)
    ident = np.eye(P, dtype=np.float32)
    maskS = np.zeros((P, NO, H), dtype=np.float32)
    maskE = np.zeros((H, NO, P), dtype=np.float32)
    invtau_pk = np.zeros((P, NO), dtype=np.float32)
    for c in range(NO):
        heads = 2 * c + pidx // HD             # [P] global head index
        maskS[pidx, c, heads] = 1.0
        maskE[heads, c, pidx] = 1.0
        invtau_pk[:, c] = inv_tau[heads]
    return ident, maskS.astype(bf16), maskE.astype(bf16), invtau_pk


def _pack_bias(b):  # b: [H, HD] -> [P, NO]
    out = np.zeros((P, NO), dtype=np.float32)
    pidx = np.arange(P)
    for c in range(NO):
        heads = 2 * c + pidx // HD
        out[:, c] = np.asarray(b, np.float32)[heads, pidx % HD]
    return out


def kernel(**inputs):
    from concourse.bass_utils import run_bass_kernel_spmd

    bf16 = ml_dtypes.bfloat16
    e4 = ml_dtypes.float8_e4m3
    x = np.ascontiguousarray(np.asarray(inputs["x"], dtype=np.float32))
    temperature = float(np.asarray(inputs["temperature"], dtype=np.float32))
    score_scale = 1.0 / (np.sqrt(HD) * temperature)

    key = round(score_scale, 12)
    if key not in _CACHE:
        _CACHE[key] = _build_nc(score_scale)
    nc = _CACHE[key]

    # weights: [H, IN, HD] -> swizzle to [NO, P, NI, P]:
    #   w[j, c, p, ic, o] = Wj[ic*128+p, c*128+o]
    wall = np.stack([
        np.asarray(inputs[n], np.float32).transpose(1, 0, 2).reshape(IN, HID)
          .reshape(NI, P, NO, P).transpose(2, 1, 0, 3)
        for n in W_NAMES
    ])                                          # [7, NO, P, NI, P] f32
    wcat8 = np.ascontiguousarray((wall * SW).astype(e4))
    wcatb = np.ascontiguousarray(wall[BF_PLANES][:, :NBF_C].astype(bf16))
    wout = np.asarray(inputs["Wout"], np.float32) * SW      # [HID, IN]
    woutb = np.ascontiguousarray(
        wout[:NBF_C * P].reshape(NBF_C, P, IN).astype(bf16))
    wout8 = np.ascontiguousarray(
        wout[NBF_C * P:].reshape(NO - NBF_C, P, IN).astype(e4))

    ident, maskS, maskE, invtau_pk = _host_consts()
    tscale = invtau_pk.copy()
    tscale[:, NBF_C:] /= SXW
    vscale = SC * tscale
    bq = _pack_bias(inputs["bq_gate"])
    bk = _pack_bias(inputs["bk_gate"])

    xr = x.reshape(N_CORES, BPC, IN)
    in_maps = []
    for c in range(N_CORES):
        xT = np.ascontiguousarray(xr[c].T).reshape(NI, P, BPC)
        in_maps.append({
            "xTb": xT.astype(bf16),
            "xT8": (xT * np.float32(SX)).astype(e4),
            "wcatb": wcatb, "wcat8": wcat8,
            "woutb": woutb, "wout8": wout8,
            "ident": ident, "maskS": maskS, "maskE": maskE,
            "tscale": tscale, "vscale": vscale, "bq": bq, "bk": bk,
        })

    res = run_bass_kernel_spmd(nc, in_maps, list(range(N_CORES)))
    kernel.last_results = res
    spike = np.concatenate(
        [np.asarray(res.results[c]["spike"], dtype=np.float32)
         for c in range(N_CORES)], axis=0)
    vnew = np.concatenate(
        [np.asarray(res.results[c]["vnew"], dtype=np.float32)
         for c in range(N_CORES)], axis=0)
    return (spike, vnew)


# revision 19
# speedup vs baseline: 1.2415x; 1.0609x over previous
"""DendriticAttentionNeuron fused Bass/Tile kernel for Trainium2 (8 NeuronCores).

Strategy: data-parallel over batch (1024 rows/core, zero collectives), with
mixed-precision matmuls. Per-head scales span 16x (invtau = 1/tau, tau
log-spaced 2..32), so the output is dominated by the large-invtau heads:
heads 0-3 (o-chunks c<2) stay bf16 while heads 4-15 run in fp8-e4m3 with
perf_mode=DoubleRow (2 contraction rows per PE cell per cycle). The gate
planes are fp8 for all heads (sigmoid attenuates their error).

Layout: transposed activations yT[out_unit, batch]; per-head constants are
per-partition scalars. Quantization scales (x*16, w*2048, v*32, wout*2048)
are exact powers of two folded into the activation scale tables, so bf16 and
fp8 contributions accumulate in consistent units everywhere.

  phase Q/K: preact chains; c<2 ff/ctx bf16 (32 mm), else fp8 DoubleRow
             (16 mm); q/k stored bf16 (c<2) / e4m3 (c>=2); scores via
             0/1-mask matmuls of q*k
  phase V:   relu(32*invtau*preact), same split
  softmax:   fp32 over the 16 heads (issued after V's matmuls so the PE
             stream stays dense while ACT/DVE do the softmax)
  phase O:   out = comb.T @ (wout*2048); co<2 bf16, co>=2 fp8 DoubleRow,
             uniform 65536x PSUM scale descaled in the spike epilogue

Measured fp32-reference max-rel-err ~1.1e-2 (gate 2e-2).
"""
import numpy as np
import ml_dtypes

B, IN, H, HD = 8192, 4096, 16, 64
HID = H * HD            # 1024
N_CORES = 8
BPC = B // N_CORES      # 1024 rows per core
P = 128
NI = IN // P            # 32 i-chunks
NO = HID // P           # 8 o-chunks
NBH = 2                 # batch halves of 512 (matmul free dim)
BH = BPC // NBH         # 512
NJS = IN // 512         # 8 output j-slices
NBC = BPC // P          # 8 batch chunks of 128
NIH = NI // 2           # 16 i-chunks per half-slab
NBF_C = 2               # o-chunks kept bf16 (heads 0..3)

SX = 16.0               # x fp8 scale
SW = 2048.0             # weight fp8 scale
SXW = SX * SW           # 32768: fp8 gemm PSUM scale
SC = 32.0               # v storage scale
SO = SC * SW            # 65536: O-phase PSUM scale

TAU_MIN, TAU_MAX, TAU_SOMA = 2.0, 32.0, 2.0
V_TH = 1.0
SURROGATE_ALPHA = 4.0

W_NAMES = ["Wq_ff", "Wq_gate", "Wq_ctx", "Wk_ff", "Wk_gate", "Wk_ctx", "Wv_ff"]
BF_PLANES = [0, 2, 3, 5, 6]   # ff/ctx/v planes have a bf16 part for c < NBF_C

_CACHE = {}


def _build_nc(score_scale: float):
    import concourse.mybir as mybir
    import concourse.tile as tile
    from concourse import bacc

    bf16 = mybir.dt.bfloat16
    f8 = mybir.dt.float8e4
    f32 = mybir.dt.float32
    AF = mybir.ActivationFunctionType
    OP = mybir.AluOpType
    DR = mybir.MatmulPerfMode.DoubleRow

    nc = bacc.Bacc("TRN2", target_bir_lowering=False, debug=False,
                   num_devices=N_CORES)

    # x pre-transposed on host: xT[ci, p, b] = x[b, ci*128+p]
    xTb_d = nc.dram_tensor("xTb", [NI, P, BPC], bf16, kind="ExternalInput").ap()
    xT8_d = nc.dram_tensor("xT8", [NI, P, BPC], f8, kind="ExternalInput").ap()
    # weights pre-swizzled on host: w[j, c, p, ic, o] = W_j[ic*128+p, c*128+o]
    wcatb_d = nc.dram_tensor("wcatb", [len(BF_PLANES), NBF_C, P, NI, P], bf16,
                             kind="ExternalInput").ap()
    wcat8_d = nc.dram_tensor("wcat8", [7, NO, P, NI, P], f8,
                             kind="ExternalInput").ap()
    woutb_d = nc.dram_tensor("woutb", [NBF_C, P, IN], bf16,
                             kind="ExternalInput").ap()
    wout8_d = nc.dram_tensor("wout8", [NO - NBF_C, P, IN], f8,
                             kind="ExternalInput").ap()
    ident_d = nc.dram_tensor("ident", [P, P], f32, kind="ExternalInput").ap()
    maskS_d = nc.dram_tensor("maskS", [P, NO, H], bf16, kind="ExternalInput").ap()
    maskE_d = nc.dram_tensor("maskE", [H, NO, P], bf16, kind="ExternalInput").ap()
    tscale_d = nc.dram_tensor("tscale", [P, NO], f32, kind="ExternalInput").ap()
    vscale_d = nc.dram_tensor("vscale", [P, NO], f32, kind="ExternalInput").ap()
    bq_d = nc.dram_tensor("bq", [P, NO], f32, kind="ExternalInput").ap()
    bk_d = nc.dram_tensor("bk", [P, NO], f32, kind="ExternalInput").ap()

    spike_d = nc.dram_tensor("spike", [BPC, IN], bf16, kind="ExternalOutput").ap()
    vnew_d = nc.dram_tensor("vnew", [BPC, IN], bf16, kind="ExternalOutput").ap()

    with tile.TileContext(nc) as tc:
        with (
            tc.tile_pool(name="const", bufs=1) as cpool,
            tc.tile_pool(name="bigx", bufs=1) as bigx,
            tc.tile_pool(name="act", bufs=1) as act,
            tc.tile_pool(name="wtsb", bufs=3) as wtsb,
            tc.tile_pool(name="wts8", bufs=5) as wts8,
            tc.tile_pool(name="wopb", bufs=2) as wopb,
            tc.tile_pool(name="wop8", bufs=6) as wop8,
            tc.tile_pool(name="tmp", bufs=4) as tmp,
            tc.tile_pool(name="prodp", bufs=2) as prodp,
            tc.tile_pool(name="outp", bufs=8) as outp,
            tc.tile_pool(name="smp", bufs=4) as smp,
            tc.tile_pool(name="ps", bufs=6, space="PSUM") as ps,
            tc.tile_pool(name="pssc", bufs=1, space="PSUM") as pssc,
        ):
            # ---- constants (gpsimd queue: keep sync free for weight slabs) ----
            ident = cpool.tile([P, P], f32)
            nc.gpsimd.dma_start(out=ident[:], in_=ident_d[:])
            maskS = cpool.tile([P, NO, H], bf16)
            nc.gpsimd.dma_start(out=maskS[:], in_=maskS_d[:])
            maskE = cpool.tile([H, NO, P], bf16)
            nc.gpsimd.dma_start(out=maskE[:], in_=maskE_d[:])
            tscale = cpool.tile([P, NO], f32)
            nc.gpsimd.dma_start(out=tscale[:], in_=tscale_d[:])
            vscale = cpool.tile([P, NO], f32)
            nc.gpsimd.dma_start(out=vscale[:], in_=vscale_d[:])
            bq = cpool.tile([P, NO], f32)
            nc.gpsimd.dma_start(out=bq[:], in_=bq_d[:])
            bk = cpool.tile([P, NO], f32)
            nc.gpsimd.dma_start(out=bk[:], in_=bk_d[:])
            spike_bias = cpool.tile([P, 1], f32)
            nc.vector.memset(spike_bias[:], float(-SURROGATE_ALPHA * V_TH))

            # ---- PE warmup: free matmuls on a zeroed tile while the first
            # DMAs are in flight, so HAM reaches K=8/8 before real work ----
            warm = cpool.tile([P, P], bf16)
            nc.vector.memset(warm[:], 0.0)
            for _ in range(48):
                wps = ps.tile([P, P], f32, tag="mm", bufs=6, name="warm")
                nc.tensor.matmul(wps[:], warm[:], warm[:], start=True,
                                 stop=True)

            # ---- load x. fp8 on the scalar queue (needed first, and few
            # enough pushes not to clog the ACT ring ahead of epilogues);
            # bf16 streams on gpsimd — only needed once c<NBF_C comes up ----
            xTb = bigx.tile([P, NI, BPC], bf16)   # 64 KiB/partition
            xT8 = bigx.tile([P, NI, BPC], f8)     # 32 KiB/partition
            for ci in range(NI):
                nc.scalar.dma_start(out=xT8[:, ci, :], in_=xT8_d[ci])
            for ci in range(NI):
                nc.gpsimd.dma_start(out=xTb[:, ci, :], in_=xTb_d[ci])

            # activation stores: bf16 part [P, NBF_C, BPC], fp8 part rest
            qTb = act.tile([P, NBF_C, BPC], bf16, tag="qb", bufs=2, name="qTb")
            kTb = act.tile([P, NBF_C, BPC], bf16, tag="qb", bufs=2, name="kTb")
            qT8 = act.tile([P, NO - NBF_C, BPC], f8, tag="q8", bufs=2, name="qT8")
            kT8 = act.tile([P, NO - NBF_C, BPC], f8, tag="q8", bufs=2, name="kT8")
            vTb = act.tile([P, NBF_C, BPC], bf16, tag="vb", bufs=1, name="vTb")
            vT8 = act.tile([P, NO - NBF_C, BPC], f8, tag="v8", bufs=1, name="vT8")

            scores = pssc.tile([H, BPC], f32, tag="sc")  # 2 PSUM banks

            # phase Q runs all-fp8 chunks first: compute starts before the
            # bf16 x stream (on the slower gpsimd queue) has arrived
            Q_ORDER = list(range(NBF_C, NO)) + list(range(NBF_C))
            NAT_ORDER = list(range(NO))

            def gemm_phase(planes, epi, c_order, first_phase=False):
                """planes: list of (j, jb); jb=None -> fp8 for all c."""
                np_ = len(planes)
                for ci_idx, c in enumerate(c_order):
                    accs = [[ps.tile([P, BH], f32, tag="mm", bufs=6,
                                     name=f"acc{jj}{bh}") for bh in range(NBH)]
                            for jj in range(np_)]
                    modes = [(jb is not None and c < NBF_C)
                             for (j, jb) in planes]
                    for half in range(2):
                        slabs = []
                        for jj, (j, jb) in enumerate(planes):
                            if modes[jj]:
                                wt = wtsb.tile([P, NIH, P], bf16, tag="wb",
                                               bufs=3, name="wtb")
                                src = wcatb_d[jb, c, :,
                                              half * NIH:(half + 1) * NIH, :]
                            else:
                                wt = wts8.tile([P, NIH, P], f8, tag="w8",
                                               bufs=5, name="wt8")
                                src = wcat8_d[j, c, :,
                                              half * NIH:(half + 1) * NIH, :]
                            if first_phase and ci_idx == 0 and half == 0:
                                for q4 in range(4):
                                    qs = slice(q4 * (NIH // 4),
                                               (q4 + 1) * (NIH // 4))
                                    nc.sync.dma_start(out=wt[:, qs, :],
                                                      in_=src[:, qs, :])
                            else:
                                nc.sync.dma_start(out=wt[:], in_=src)
                            slabs.append(wt)
                        # bh-middle order: bh0's accumulation finishes before
                        # bh1's second-half matmuls, so its epilogue overlaps
                        # them and frees PSUM banks for the next c
                        for bh in range(NBH):
                            sl = slice(bh * BH, (bh + 1) * BH)
                            for t in range(NIH // 2):
                                for sub in range(2):
                                    il = 2 * t + sub
                                    i = half * NIH + il
                                    for jj in range(np_):
                                        if modes[jj]:
                                            nc.tensor.matmul(
                                                accs[jj][bh][:],
                                                slabs[jj][:, il, :],
                                                xTb[:, i, sl],
                                                start=(i == 0),
                                                stop=(i == NI - 1))
                                i0 = half * NIH + 2 * t
                                for jj in range(np_):
                                    if not modes[jj]:
                                        nc.tensor.matmul(
                                            accs[jj][bh][:],
                                            slabs[jj][:, 2 * t:2 * t + 2, :],
                                            xT8[:, i0:i0 + 2, sl],
                                            start=(i0 == 0),
                                            stop=(i0 + 2 == NI),
                                            perf_mode=DR)
                            if half == 1:
                                epi(c, bh,
                                    [accs[jj][bh] for jj in range(np_)])

            # ---- phase Q ----
            def make_qk_epi(dstb, dst8, bias, with_scores):
                def epi(c, bh, acc):
                    ff, gg, cc = acc
                    sl = slice(bh * BH, (bh + 1) * BH)
                    sig = tmp.tile([P, BH], f32, tag="t", bufs=4, name="sig")
                    nc.scalar.activation(sig[:], gg[:], AF.Sigmoid,
                                         bias=bias[:, c:c + 1],
                                         scale=float(1.0 / SXW))
                    t1 = tmp.tile([P, BH], f32, tag="t", bufs=4, name="t1")
                    nc.vector.tensor_tensor(out=t1[:], in0=sig[:], in1=cc[:],
                                            op=OP.mult)
                    t2 = tmp.tile([P, BH], f32, tag="t", bufs=4, name="t2")
                    nc.vector.tensor_tensor(out=t2[:], in0=t1[:], in1=ff[:],
                                            op=OP.add)
                    dst = dstb[:, c, sl] if c < NBF_C else dst8[:, c - NBF_C, sl]
                    nc.scalar.activation(dst, t2[:], AF.Tanh,
                                         scale=tscale[:, c:c + 1])
                    if with_scores:
                        qsrc = (qTb[:, c, sl] if c < NBF_C
                                else qT8[:, c - NBF_C, sl])
                        ksrc = (kTb[:, c, sl] if c < NBF_C
                                else kT8[:, c - NBF_C, sl])
                        prod = prodp.tile([P, BH], bf16, tag="p", bufs=2,
                                          name="prod")
                        nc.vector.tensor_tensor(out=prod[:], in0=qsrc,
                                                in1=ksrc, op=OP.mult)
                        nc.tensor.matmul(scores[:, sl], maskS[:, c, :],
                                         prod[:], start=(c == 0),
                                         stop=(c == NO - 1))
                return epi

            gemm_phase([(0, 0), (1, None), (2, 1)],
                       make_qk_epi(qTb, qT8, bq, False), Q_ORDER,
                       first_phase=True)

            # ---- phase K (+ score accumulation) ----
            gemm_phase([(3, 2), (4, None), (5, 3)],
                       make_qk_epi(kTb, kT8, bk, True), NAT_ORDER)

            # ---- softmax over heads (PE transposes run before V's matmuls,
            # the ACT/DVE chain overlaps them) ----
            scores_sb = smp.tile([H, BPC], f32, tag="ssb", bufs=1)
            nc.scalar.activation(scores_sb[:], scores[:], AF.Copy,
                                 scale=float(score_scale))
            attnT = smp.tile([H, BPC], bf16, tag="att", bufs=1)
            # two dense PE loops (all forward transposes, then all backward)
            # so the PE never dribbles waiting on the per-chunk ACT/DVE chain
            ats = []
            for bt in range(NBC):
                sl = slice(bt * P, (bt + 1) * P)
                tp = ps.tile([P, H], f32, tag="mm", bufs=6, name="tp")
                nc.tensor.transpose(tp[:], scores_sb[:, sl], ident[:H, :H])
                ex = smp.tile([P, H], f32, tag="sm", bufs=4, name="ex")
                nc.scalar.activation(ex[:], tp[:], AF.Exp)
                ssum = smp.tile([P, 1], f32, tag="sms", bufs=4, name="ssum")
                nc.vector.reduce_sum(out=ssum[:], in_=ex[:],
                                     axis=mybir.AxisListType.X)
                rec = smp.tile([P, 1], f32, tag="sms", bufs=4, name="rec")
                nc.vector.reciprocal(rec[:], ssum[:])
                at = smp.tile([P, H], f32, tag="atl", bufs=NBC,
                              name=f"at{bt}")
                nc.vector.tensor_scalar_mul(at[:], ex[:], rec[:])
                ats.append(at)
            for bt in range(NBC):
                sl = slice(bt * P, (bt + 1) * P)
                tp2 = ps.tile([H, P], f32, tag="mm", bufs=6, name="tp2")
                nc.tensor.transpose(tp2[:], ats[bt][:], ident[:])
                nc.vector.tensor_copy(attnT[:, sl], tp2[:])

            # ---- phase V, with attn-expand + combine fused into the
            # epilogue so comb is ready the moment the last V matmul ends ----
            combb = act.tile([P, NBF_C, BPC], bf16, tag="qb", bufs=2,
                             name="combb")
            comb8 = act.tile([P, NO - NBF_C, BPC], f8, tag="q8", bufs=2,
                             name="comb8")

            def epi_v(c, bh, acc):
                sl = slice(bh * BH, (bh + 1) * BH)
                dst = vTb[:, c, sl] if c < NBF_C else vT8[:, c - NBF_C, sl]
                nc.scalar.activation(dst, acc[0][:], AF.Relu,
                                     scale=vscale[:, c:c + 1])
                exp_ps = ps.tile([P, BH], f32, tag="mm", bufs=6, name="expps")
                nc.tensor.matmul(exp_ps[:], maskE[:, c, :], attnT[:, sl],
                                 start=True, stop=True)
                cdst = (combb[:, c, sl] if c < NBF_C
                        else comb8[:, c - NBF_C, sl])
                nc.vector.tensor_tensor(out=cdst, in0=exp_ps[:], in1=dst,
                                        op=OP.mult)

            def load_wo(jp, eng):
                wob = wopb.tile([P, 2, 1024], bf16, tag="wob", bufs=2,
                                name="wob")
                for col in range(2):
                    eng.dma_start(
                        out=wob[:, col, :],
                        in_=woutb_d[col, :, jp * 1024:(jp + 1) * 1024])
                wo8s = []
                for tp8 in range(3):
                    wo8 = wop8.tile([P, 2, 1024], f8, tag="wo8", bufs=6,
                                    name="wo8")
                    for col in range(2):
                        eng.dma_start(
                            out=wo8[:, col, :],
                            in_=wout8_d[tp8 * 2 + col, :,
                                        jp * 1024:(jp + 1) * 1024])
                    wo8s.append(wo8)
                return wob, wo8s

            # jp=0 wout prefetch on the idle gpsimd queue, before phase V
            wo_tiles = load_wo(0, nc.gpsimd)

            gemm_phase([(6, 4)], epi_v, NAT_ORDER)

            # ---- phase O: output projection + spike/v_new ----
            for jp in range(NJS // 2):
                wob, wo8s = wo_tiles
                if jp + 1 < NJS // 2:
                    wo_tiles = load_wo(jp + 1, nc.sync)
                for bc in range(NBC):
                    bsl = slice(bc * P, (bc + 1) * P)
                    pos = [ps.tile([P, 512], f32, tag="mm", bufs=6,
                                   name=f"po{h}") for h in range(2)]
                    for step in range(5):
                        for h in range(2):
                            hsl = slice(h * 512, (h + 1) * 512)
                            if step < NBF_C:
                                nc.tensor.matmul(
                                    pos[h][:], combb[:, step, bsl],
                                    wob[:, step, hsl],
                                    start=(step == 0), stop=False)
                            else:
                                tp8 = step - NBF_C
                                nc.tensor.matmul(
                                    pos[h][:],
                                    comb8[:, 2 * tp8:2 * tp8 + 2, bsl],
                                    wo8s[tp8][:, :, hsl],
                                    start=False, stop=(step == 4),
                                    perf_mode=DR)
                    for h in range(2):
                        js = jp * 2 + h
                        po = pos[h]
                        spk = outp.tile([P, 512], bf16, tag="o", bufs=8,
                                        name="spk")
                        nc.scalar.activation(
                            spk[:], po[:], AF.Sigmoid,
                            scale=float(SURROGATE_ALPHA / (TAU_SOMA * SO)),
                            bias=spike_bias[:])
                        vnw = outp.tile([P, 512], bf16, tag="o", bufs=8,
                                        name="vnw")
                        nc.vector.scalar_tensor_tensor(
                            out=vnw[:], in0=po[:],
                            scalar=float(1.0 / (TAU_SOMA * SO)),
                            in1=spk[:], op0=OP.mult, op1=OP.subtract)
                        nc.scalar.dma_start(
                            out=spike_d[bsl, js * 512:(js + 1) * 512],
                            in_=spk[:])
                        nc.sync.dma_start(
                            out=vnew_d[bsl, js * 512:(js + 1) * 512],
                            in_=vnw[:])

    nc.finalize()
    return nc


def _host_consts():
    bf16 = ml_dtypes.bfloat16
    taus = np.logspace(np.log10(TAU_MIN), np.log10(TAU_MAX), H).astype(np.float32)
    inv_tau = 1.0 / taus                       # [H]
    pidx = np.arange(P)
    ident = np.eye(P, dtype=np.float32)
    maskS = np.zeros((P, NO, H), dtype=np.float32)
    maskE = np.zeros((H, NO, P), dtype=np.float32)
    invtau_pk = np.zeros((P, NO), dtype=np.float32)
    for c in range(NO):
        heads = 2 * c + pidx // HD             # [P] global head index
        maskS[pidx, c, heads] = 1.0
        maskE[heads, c, pidx] = 1.0
        invtau_pk[:, c] = inv_tau[heads]
    return ident, maskS.astype(bf16), maskE.astype(bf16), invtau_pk


def _pack_bias(b):  # b: [H, HD] -> [P, NO]
    out = np.zeros((P, NO), dtype=np.float32)
    pidx = np.arange(P)
    for c in range(NO):
        heads = 2 * c + pidx // HD
        out[:, c] = np.asarray(b, np.float32)[heads, pidx % HD]
    return out


def kernel(**inputs):
    from concourse.bass_utils import run_bass_kernel_spmd

    bf16 = ml_dtypes.bfloat16
    e4 = ml_dtypes.float8_e4m3
    x = np.ascontiguousarray(np.asarray(inputs["x"], dtype=np.float32))
    temperature = float(np.asarray(inputs["temperature"], dtype=np.float32))
    score_scale = 1.0 / (np.sqrt(HD) * temperature)

    key = round(score_scale, 12)
    if key not in _CACHE:
        _CACHE[key] = _build_nc(score_scale)
    nc = _CACHE[key]

    # weights: [H, IN, HD] -> swizzle to [NO, P, NI, P]:
    #   w[j, c, p, ic, o] = Wj[ic*128+p, c*128+o]
    wall = np.stack([
        np.asarray(inputs[n], np.float32).transpose(1, 0, 2).reshape(IN, HID)
          .reshape(NI, P, NO, P).transpose(2, 1, 0, 3)
        for n in W_NAMES
    ])                                          # [7, NO, P, NI, P] f32
    wcat8 = np.ascontiguousarray((wall * SW).astype(e4))
    wcatb = np.ascontiguousarray(wall[BF_PLANES][:, :NBF_C].astype(bf16))
    wout = np.asarray(inputs["Wout"], np.float32) * SW      # [HID, IN]
    woutb = np.ascontiguousarray(
        wout[:NBF_C * P].reshape(NBF_C, P, IN).astype(bf16))
    wout8 = np.ascontiguousarray(
        wout[NBF_C * P:].reshape(NO - NBF_C, P, IN).astype(e4))

    ident, maskS, maskE, invtau_pk = _host_consts()
    tscale = invtau_pk.copy()
    tscale[:, NBF_C:] /= SXW
    vscale = SC * tscale
    bq = _pack_bias(inputs["bq_gate"])
    bk = _pack_bias(inputs["bk_gate"])

    xr = x.reshape(N_CORES, BPC, IN)
    in_maps = []
    for c in range(N_CORES):
        xT = np.ascontiguousarray(xr[c].T).reshape(NI, P, BPC)
        in_maps.append({
            "xTb": xT.astype(bf16),
            "xT8": (xT * np.float32(SX)).astype(e4),
            "wcatb": wcatb, "wcat8": wcat8,
            "woutb": woutb, "wout8": wout8,
            "ident": ident, "maskS": maskS, "maskE": maskE,
            "tscale": tscale, "vscale": vscale, "bq": bq, "bk": bk,
        })

    res = run_bass_kernel_spmd(nc, in_maps, list(range(N_CORES)))
    kernel.last_results = res
    spike = np.concatenate(
        [np.asarray(res.results[c]["spike"], dtype=np.float32)
         for c in range(N_CORES)], axis=0)
    vnew = np.concatenate(
        [np.asarray(res.results[c]["vnew"], dtype=np.float32)
         for c in range(N_CORES)], axis=0)
    return (spike, vnew)
